# revision 6
# baseline (speedup 1.0000x reference)
"""Trainium2 Bass kernel for nn_DULLI_21869973471277 (vq_codebook).

Data-parallel over batch across 8 NeuronCores. Each core handles 4 of 32
batch items (6400 of 51200 tokens):

  VQ0: d0 = |x|^2 + |e|^2 - 2 x.e  (k=2048, c=64) -> softmax(-d0/0.1) ->
       qx0 = gate*x + (1-gate) p0 @ E0
  head: x3 = W2 relu(W1 qx0 + b1) + b2   (1x1 convs, c 64->256->256)
  VQ1: same with k=512, c=256 -> qx3, a3, d3

Layouts: distances computed in [token, k] tiles (tokens on partitions) so
softmax reduces along the free axis; probabilities are PE-transposed into
[k, token] blocks which are simultaneously the a0/a3 output layout and the
lhs-contraction layout for q = p @ E.

All distance/feature matmuls use an fp16 hi/lo split (v = hi + lo, each
fp16): 3 bf16-speed matmuls replace one fp32 matmul (4x slower per row)
while keeping ~2^-22 relative error, which the T=0.1 softmax demands
(plain fp16/bf16/fp32r all fail: a d-error of eps becomes a factor
exp(10*eps) on the probabilities). The |x|^2-style norm rows ride in the
same augmented contraction and only need loose (distance-output) accuracy,
so they stay single fp16 rows; the |e|^2 rows vary along k and get hi+lo.
"""
from contextlib import ExitStack

import numpy as np

import concourse.bass as bass
import concourse.mybir as mybir
import concourse.tile as tile
from concourse import bacc
from concourse.bass_utils import run_bass_kernel_spmd
from concourse.masks import make_identity

F32 = mybir.dt.float32
F16 = mybir.dt.float16
AF = mybir.ActivationFunctionType
AX = mybir.AxisListType
ALU = mybir.AluOpType

B, C, H, W = 32, 64, 40, 40
HW = H * W                      # 1600 tokens per batch item
K0, C1, K1 = 2048, 256, 512
NCORES = 8
BL = B // NCORES                # 4 batch items per core
NL = BL * HW                    # 6400 tokens per core
TEMP = 0.1
P = 128
LAST_EXEC_NS = None
SUP = 256                       # super-tile: 2 sub-tiles of 128 tokens


def _pieces(tok0, n, hw):
    """Split core-local token range [tok0, tok0+n) into per-batch-item
    (b, hw0, col_off, width) pieces for strided output DMAs."""
    out = []
    t = tok0
    while t < tok0 + n:
        b = t // hw
        hw0 = t % hw
        w = min(hw - hw0, tok0 + n - t)
        out.append((b, hw0, t - tok0, w))
        t += w
    return out


def _split16(nc, hi, lo, src):
    """hi = fp16(src); lo = fp16(src - hi). src f32, hi/lo fp16 APs."""
    nc.vector.tensor_copy(hi, src)
    nc.vector.tensor_sub(lo, src, hi)


def build_nc(gate, bl=BL, hw=HW):
    nl = bl * hw
    assert nl % SUP == 0
    nsup = nl // SUP
    g1 = 1.0 - gate
    tK0 = K0 // P               # 16 k-chunks for VQ0
    tK1 = K1 // P               # 4 k-chunks for VQ1
    nC1 = C1 // P               # 2 channel chunks for c=256

    nc = bacc.Bacc()
    x0p = nc.declare_dram_parameter("x0", [bl, C, hw], F32, isOutput=False)
    vq0p = nc.declare_dram_parameter("vq0", [K0, C], F32, isOutput=False)
    vq1p = nc.declare_dram_parameter("vq1", [K1, C1], F32, isOutput=False)
    w1p = nc.declare_dram_parameter("head_w1", [C1, C], F32, isOutput=False)
    b1p = nc.declare_dram_parameter("head_b1", [nC1, P, 1], F32, isOutput=False)
    w2p = nc.declare_dram_parameter("head_w2", [C1, C1], F32, isOutput=False)
    b2p = nc.declare_dram_parameter("head_b2", [nC1, P, 1], F32, isOutput=False)

    x3p = nc.declare_dram_parameter("x3", [bl, C1, hw], F32, isOutput=True)
    qx0p = nc.declare_dram_parameter("qx0", [bl, C, hw], F32, isOutput=True)
    qx3p = nc.declare_dram_parameter("qx3", [bl, C1, hw], F32, isOutput=True)
    a0p = nc.declare_dram_parameter("a0", [bl, K0, hw], F32, isOutput=True)
    a3p = nc.declare_dram_parameter("a3", [bl, K1, hw], F32, isOutput=True)
    d0p = nc.declare_dram_parameter("d0", [nl, K0], F32, isOutput=True)
    d3p = nc.declare_dram_parameter("d3", [nl, K1], F32, isOutput=True)

    with tile.TileContext(nc) as tc, ExitStack() as ctx:
        pers = ctx.enter_context(tc.tile_pool(name="pers", bufs=1))
        ps = ctx.enter_context(tc.tile_pool(name="ps", bufs=1, space="PSUM"))

        ident = pers.tile([P, P], F32)
        make_identity(nc, ident)
        ones_c = pers.tile([C, 1], F32)
        nc.gpsimd.memset(ones_c, 1.0)
        ones_p = pers.tile([P, 1], F32)
        nc.gpsimd.memset(ones_p, 1.0)
        ones_p16 = pers.tile([P, 1], F16)
        nc.gpsimd.memset(ones_p16, 1.0)

        # ---- persistent operand tiles -------------------------------------
        xa_hi = pers.tile([C + 2, nl], F16)   # rows: x | 1 | |x|^2
        xa_lo = pers.tile([C + 2, nl], F16)
        v0r_hi = pers.tile([C + 2, K0], F16)  # rows: -2 E^T | |e|^2 | 1
        v0r_lo = pers.tile([C + 2, K0], F16)
        v0g_hi = pers.tile([P, tK0, C], F16)  # (1-gate) * E, natural layout
        v0g_lo = pers.tile([P, tK0, C], F16)
        v1r_hi = [pers.tile([P, K1], F16, name=f"v1rh{g}", tag=f"v1rh{g}") for g in range(nC1)]
        v1r_lo = [pers.tile([P, K1], F16, name=f"v1rl{g}", tag=f"v1rl{g}") for g in range(nC1)]
        v1aug = pers.tile([3, K1], F16)       # rows: 1 | |e1|^2 hi | |e1|^2 lo
        xn_row = pers.tile([1, nl], F16)      # staging for the |x|^2 row
        v1g_hi = pers.tile([P, tK1, C1], F16)
        w1t = pers.tile([C, C1], F32)
        w2t = [pers.tile([P, C1], F32, name=f"w2t{g}", tag=f"w2t{g}") for g in range(nC1)]
        b1t = [pers.tile([P, 1], F32, name=f"b1t{g}", tag=f"b1t{g}") for g in range(nC1)]
        b2t = [pers.tile([P, 1], F32, name=f"b2t{g}", tag=f"b2t{g}") for g in range(nC1)]
        x3aug = pers.tile([3, nl], F16)       # rows: |x3|^2 | 1 | 1

        nc.gpsimd.memset(xa_hi[C:C + 2, :], 1.0)   # row C+1 redone via DMA
        nc.gpsimd.memset(xa_lo[C:C + 2, :], 0.0)
        nc.gpsimd.memset(v0r_hi[C:C + 2, :], 1.0)  # row C overwritten below
        nc.gpsimd.memset(v0r_lo[C:C + 2, :], 0.0)
        nc.gpsimd.memset(v1aug, 1.0)               # rows 1,2 redone via DMA
        nc.gpsimd.memset(x3aug, 1.0)               # row 0 rewritten per-super

        for g in range(nC1):
            nc.sync.dma_start(out=b1t[g], in_=b1p[g])
            nc.sync.dma_start(out=b2t[g], in_=b2p[g])

        # ---- setup: transposes, norms, fp16 splits ------------------------
        with tc.tile_pool(name="setup", bufs=1) as sp:
            x_f = sp.tile([C, nl], F32)
            for b in range(bl):
                nc.sync.dma_start(out=x_f[:, b * hw:(b + 1) * hw], in_=x0p[b])
            _split16(nc, xa_hi[0:C, :], xa_lo[0:C, :], x_f)
            for cstart in range(0, nl, 512):
                w = min(512, nl - cstart)
                xsq = sp.tile([C, 512], F32, tag="xsq")
                nc.vector.tensor_mul(xsq[:, :w], x_f[:, cstart:cstart + w],
                                     x_f[:, cstart:cstart + w])
                ps_r = ps.tile([1, 512], F32, tag="misc")
                nc.tensor.matmul(ps_r[:, :w], ones_c, xsq[:, :w],
                                 start=True, stop=True)
                nc.scalar.copy(xn_row[:, cstart:cstart + w], ps_r[:, :w])

            nc.sync.dma_start(out=xa_hi[C + 1:C + 2, :], in_=xn_row)

            # VQ0 codebook: natural load, transpose to [c, k], scale -2
            v0nat = sp.tile([P, tK0, C], F32)
            nc.sync.dma_start(out=v0nat,
                              in_=vq0p.rearrange("(t p) c -> p t c", p=P))
            v0t = sp.tile([C, K0], F32)     # holds -2 E^T
            for t in range(tK0):
                ps_t = ps.tile([P, P], F32, tag="pt")
                nc.tensor.transpose(ps_t[:C, :], v0nat[:, t, :], ident)
                nc.vector.tensor_scalar_mul(v0t[:, t * P:(t + 1) * P],
                                            ps_t[:C, :], -2.0)
            _split16(nc, v0r_hi[0:C, :], v0r_lo[0:C, :], v0t)
            en_hi0 = sp.tile([1, K0], F16)
            en_lo0 = sp.tile([1, K0], F16)
            for cstart in range(0, K0, 512):
                cs = slice(cstart, cstart + 512)
                esq = sp.tile([C, 512], F32, tag="esq")
                nc.vector.tensor_mul(esq, v0t[:, cs], v0t[:, cs])
                ps_r = ps.tile([1, 512], F32, tag="misc")
                nc.tensor.matmul(ps_r, ones_c, esq, start=True, stop=True)
                en_f = sp.tile([1, 512], F32, tag="enf")
                nc.vector.tensor_scalar_mul(en_f, ps_r, 0.25)
                nc.vector.tensor_copy(en_hi0[:, cs], en_f)
                nc.vector.tensor_sub(en_lo0[:, cs], en_f, en_hi0[:, cs])
            nc.sync.dma_start(out=v0r_hi[C:C + 1, :], in_=en_hi0)
            nc.sync.dma_start(out=v0r_lo[C:C + 1, :], in_=en_lo0)

            vgf = sp.tile([P, tK0, C], F32)
            nc.vector.tensor_scalar_mul(vgf, v0nat, g1)
            _split16(nc, v0g_hi, v0g_lo, vgf)

            # VQ1 codebook
            v1nat = sp.tile([P, tK1, C1], F32)
            nc.sync.dma_start(out=v1nat,
                              in_=vq1p.rearrange("(t p) c -> p t c", p=P))
            v1t = [sp.tile([P, K1], F32, name=f"v1t{g}", tag=f"v1t{g}") for g in range(nC1)]
            for t in range(tK1):
                for g in range(nC1):
                    ps_t = ps.tile([P, P], F32, tag="pt")
                    nc.tensor.transpose(ps_t, v1nat[:, t, g * P:(g + 1) * P], ident)
                    nc.vector.tensor_scalar_mul(v1t[g][:, t * P:(t + 1) * P],
                                                ps_t, -2.0)
            for g in range(nC1):
                _split16(nc, v1r_hi[g], v1r_lo[g], v1t[g])
            ps_r = ps.tile([1, 512], F32, tag="misc")
            for g in range(nC1):
                esq = sp.tile([P, K1], F32, tag="esq1")
                nc.vector.tensor_mul(esq, v1t[g], v1t[g])
                nc.tensor.matmul(ps_r, ones_p, esq, start=(g == 0),
                                 stop=(g == nC1 - 1))
            en_f = sp.tile([1, 512], F32, tag="enf")
            nc.vector.tensor_scalar_mul(en_f, ps_r, 0.25)
            en1_hi = sp.tile([1, 512], F16)
            en1_lo = sp.tile([1, 512], F16)
            nc.vector.tensor_copy(en1_hi, en_f)
            nc.vector.tensor_sub(en1_lo, en_f, en1_hi)
            nc.sync.dma_start(out=v1aug[1:2, :], in_=en1_hi)
            nc.sync.dma_start(out=v1aug[2:3, :], in_=en1_lo)

            vg1f = sp.tile([P, tK1, C1], F32)
            nc.vector.tensor_scalar_mul(vg1f, v1nat, g1)
            nc.vector.tensor_copy(v1g_hi, vg1f)

            # head weights: transpose W1 (256,64) -> [64, 256], W2 (256,256)
            w1nat = sp.tile([P, nC1, C], F32)
            nc.sync.dma_start(out=w1nat,
                              in_=w1p.rearrange("(t p) c -> p t c", p=P))
            for t in range(nC1):
                ps_t = ps.tile([P, P], F32, tag="pt")
                nc.tensor.transpose(ps_t[:C, :], w1nat[:, t, :], ident)
                nc.scalar.copy(w1t[:, t * P:(t + 1) * P], ps_t[:C, :])
            w2nat = sp.tile([P, nC1, C1], F32)
            nc.sync.dma_start(out=w2nat,
                              in_=w2p.rearrange("(t p) c -> p t c", p=P))
            for t in range(nC1):
                for g in range(nC1):
                    ps_t = ps.tile([P, P], F32, tag="pt")
                    nc.tensor.transpose(ps_t, w2nat[:, t, g * P:(g + 1) * P], ident)
                    nc.scalar.copy(w2t[g][:, t * P:(t + 1) * P], ps_t)

        # ---- main loop ----------------------------------------------------
        work = ctx.enter_context(tc.tile_pool(name="work", bufs=1))
        dbuf = ctx.enter_context(tc.tile_pool(name="dbuf", bufs=2))

        for s in range(nsup):
            t0 = s * SUP
            pt_f = work.tile([P, tK0, SUP], F32, tag="pt_f")
            pt_hi = work.tile([P, tK0, SUP], F16, tag="pt_hi")
            pt_lo = work.tile([P, tK0, SUP], F16, tag="pt_lo")

            for sub in range(2):
                tok = t0 + sub * P
                sl = slice(tok, tok + P)
                # d0 = |x|^2 + |e|^2 - 2 x.e via augmented split matmuls
                ps_d = ps.tile([P, K0], F32, tag="d0")
                for lhs, rhs, st, sp_ in ((xa_hi, v0r_hi, True, False),
                                          (xa_hi, v0r_lo, False, False),
                                          (xa_lo, v0r_hi, False, True)):
                    for j in range(K0 // 512):
                        js = slice(j * 512, (j + 1) * 512)
                        nc.tensor.matmul(ps_d[:, js], lhs[:, sl], rhs[:, js],
                                         start=st, stop=sp_)
                rmn = dbuf.tile([P, 1], F32, tag="rmn")
                nc.vector.tensor_reduce(rmn, ps_d, axis=AX.X, op=ALU.min,
                                        negate=True)
                bias10 = dbuf.tile([P, 1], F32, tag="b10")
                nc.vector.tensor_scalar_mul(bias10, rmn, -1.0 / TEMP)
                d_sb = dbuf.tile([P, K0], F32, tag="d_sb")
                nc.any.tensor_copy(d_sb, ps_d)
                nc.sync.dma_start(out=d0p[sl, :], in_=d_sb)
                e0 = dbuf.tile([P, K0], F32, tag="e0")
                rs = dbuf.tile([P, 1], F32, tag="rs")
                nc.scalar.activation(e0, ps_d, AF.Exp, bias=bias10,
                                     scale=-1.0 / TEMP, accum_out=rs)
                rinv = dbuf.tile([P, 1], F32, tag="rinv")
                nc.vector.reciprocal(rinv, rs)
                nc.vector.tensor_scalar_mul(e0, e0, rinv)   # e0 <- p0
                ssl = slice(sub * P, (sub + 1) * P)
                for j in range(tK0):
                    ps_t = ps.tile([P, P], F32, tag="pt")
                    nc.tensor.transpose(ps_t, e0[:, j * P:(j + 1) * P], ident)
                    nc.any.tensor_copy(pt_f[:, j, ssl], ps_t)
                    nc.vector.tensor_copy(pt_hi[:, j, ssl], ps_t)
                    nc.vector.tensor_sub(pt_lo[:, j, ssl], ps_t, pt_hi[:, j, ssl])

            pcs = _pieces(t0, SUP, hw)
            for j in range(tK0):
                for (b, hw0, co, w) in pcs:
                    nc.sync.dma_start(out=a0p[b, j * P:(j + 1) * P, hw0:hw0 + w],
                                      in_=pt_f[:, j, co:co + w])

            # qx0 = gate*x + (1-gate) p0 @ E0   (computed transposed [C, SUP])
            ps_q = ps.tile([C, SUP], F32, tag="misc")
            for t in range(tK0):
                nc.tensor.matmul(ps_q, v0g_hi[:, t, :], pt_hi[:, t, :],
                                 start=(t == 0), stop=False)
                nc.tensor.matmul(ps_q, v0g_hi[:, t, :], pt_lo[:, t, :],
                                 start=False, stop=False)
                nc.tensor.matmul(ps_q, v0g_lo[:, t, :], pt_hi[:, t, :],
                                 start=False, stop=(t == tK0 - 1))
            ssup = slice(t0, t0 + SUP)
            qx0 = dbuf.tile([C, SUP], F32, tag="qx0")
            nc.vector.tensor_add(qx0, xa_hi[0:C, ssup], xa_lo[0:C, ssup])
            nc.vector.tensor_scalar_mul(qx0, qx0, gate)
            nc.vector.tensor_add(qx0, qx0, ps_q)
            for (b, hw0, co, w) in pcs:
                nc.sync.dma_start(out=qx0p[b, :, hw0:hw0 + w],
                                  in_=qx0[:, co:co + w])

            # head: y1 = relu(W1 qx0 + b1); x3 = W2 y1 + b2
            y1 = [dbuf.tile([P, SUP], F32, name=f"y1_{g}", tag=f"y1_{g}") for g in range(nC1)]
            for g in range(nC1):
                ps_y = ps.tile([P, SUP], F32, tag="misc")
                nc.tensor.matmul(ps_y, w1t[:, g * P:(g + 1) * P], qx0,
                                 start=True, stop=True)
                nc.scalar.activation(y1[g], ps_y, AF.Relu, bias=b1t[g])
            x3t = [dbuf.tile([P, SUP], F32, name=f"x3_{o}", tag=f"x3_{o}") for o in range(nC1)]
            x3hi = [dbuf.tile([P, SUP], F16, name=f"x3h{o}", tag=f"x3h{o}") for o in range(nC1)]
            x3lo = [dbuf.tile([P, SUP], F16, name=f"x3l{o}", tag=f"x3l{o}") for o in range(nC1)]
            for o in range(nC1):
                ps_x = ps.tile([P, SUP], F32, tag="misc")
                for g in range(nC1):
                    nc.tensor.matmul(ps_x, w2t[g][:, o * P:(o + 1) * P], y1[g],
                                     start=(g == 0), stop=(g == nC1 - 1))
                nc.scalar.activation(x3t[o], ps_x, AF.Identity, bias=b2t[o])
                for (b, hw0, co, w) in pcs:
                    nc.sync.dma_start(out=x3p[b, o * P:(o + 1) * P, hw0:hw0 + w],
                                      in_=x3t[o][:, co:co + w])
                _split16(nc, x3hi[o], x3lo[o], x3t[o])

            # |x3|^2 row (fp16 accuracy is plenty: uniform across k)
            ps_r = ps.tile([1, SUP], F32, tag="misc")
            for o in range(nC1):
                sq = dbuf.tile([P, SUP], F16, tag="sq3")
                nc.vector.tensor_mul(sq, x3t[o], x3t[o])
                nc.tensor.matmul(ps_r, ones_p16, sq, start=(o == 0),
                                 stop=(o == nC1 - 1))
            nc.scalar.copy(x3aug[0:1, ssup], ps_r)

            # VQ1 distance + softmax per sub-tile
            pt3_f = work.tile([P, tK1, SUP], F32, tag="pt3_f")
            pt3_hi = work.tile([P, tK1, SUP], F16, tag="pt3_hi")
            for sub in range(2):
                tok = t0 + sub * P
                sl = slice(tok, tok + P)
                ssl = slice(sub * P, (sub + 1) * P)
                ps_d3 = ps.tile([P, K1], F32, tag="misc")
                for g in range(nC1):
                    nc.tensor.matmul(ps_d3, x3hi[g][:, ssl], v1r_hi[g],
                                     start=(g == 0), stop=False)
                    nc.tensor.matmul(ps_d3, x3hi[g][:, ssl], v1r_lo[g],
                                     start=False, stop=False)
                for g in range(nC1):
                    nc.tensor.matmul(ps_d3, x3lo[g][:, ssl], v1r_hi[g],
                                     start=False, stop=False)
                nc.tensor.matmul(ps_d3, x3aug[:, sl], v1aug,
                                 start=False, stop=True)
                rmn = dbuf.tile([P, 1], F32, tag="rmn3")
                nc.vector.tensor_reduce(rmn, ps_d3, axis=AX.X, op=ALU.min,
                                        negate=True)
                bias10 = dbuf.tile([P, 1], F32, tag="b103")
                nc.vector.tensor_scalar_mul(bias10, rmn, -1.0 / TEMP)
                d_sb = dbuf.tile([P, K1], F32, tag="d3_sb")
                nc.scalar.copy(d_sb, ps_d3)
                nc.sync.dma_start(out=d3p[sl, :], in_=d_sb)
                e3 = dbuf.tile([P, K1], F32, tag="e3")
                rs = dbuf.tile([P, 1], F32, tag="rs3")
                nc.scalar.activation(e3, ps_d3, AF.Exp, bias=bias10,
                                     scale=-1.0 / TEMP, accum_out=rs)
                rinv = dbuf.tile([P, 1], F32, tag="rinv3")
                nc.vector.reciprocal(rinv, rs)
                nc.vector.tensor_scalar_mul(e3, e3, rinv)   # e3 <- p3
                for t in range(tK1):
                    ps_t = ps.tile([P, P], F32, tag="pt")
                    nc.tensor.transpose(ps_t, e3[:, t * P:(t + 1) * P], ident)
                    nc.any.tensor_copy(pt3_f[:, t, ssl], ps_t)
                    nc.vector.tensor_copy(pt3_hi[:, t, ssl], ps_t)

            for t in range(tK1):
                for (b, hw0, co, w) in pcs:
                    nc.sync.dma_start(out=a3p[b, t * P:(t + 1) * P, hw0:hw0 + w],
                                      in_=pt3_f[:, t, co:co + w])

            # qx3 = gate*x3 + (1-gate) p3 @ E1
            for o in range(nC1):
                ps_q3 = ps.tile([P, SUP], F32, tag="misc")
                for t in range(tK1):
                    nc.tensor.matmul(ps_q3, v1g_hi[:, t, o * P:(o + 1) * P],
                                     pt3_hi[:, t, :],
                                     start=(t == 0), stop=(t == tK1 - 1))
                qx3 = dbuf.tile([P, SUP], F32, tag=f"qx3_{o}")
                nc.vector.tensor_scalar_mul(qx3, x3t[o], gate)
                nc.vector.tensor_add(qx3, qx3, ps_q3)
                for (b, hw0, co, w) in pcs:
                    nc.sync.dma_start(out=qx3p[b, o * P:(o + 1) * P, hw0:hw0 + w],
                                      in_=qx3[:, co:co + w])

    nc.compile()
    return nc


def kernel(**inputs):
    x0 = np.ascontiguousarray(np.asarray(inputs["x0"], dtype=np.float32)
                              .reshape(B, C, HW))
    vq0 = np.ascontiguousarray(np.asarray(inputs["vq0"], dtype=np.float32))
    vq1 = np.ascontiguousarray(np.asarray(inputs["vq1"], dtype=np.float32))
    w1 = np.ascontiguousarray(np.asarray(inputs["head_w1"], dtype=np.float32))
    b1 = np.ascontiguousarray(np.asarray(inputs["head_b1"], dtype=np.float32)
                              .reshape(C1 // P, P, 1))
    w2 = np.ascontiguousarray(np.asarray(inputs["head_w2"], dtype=np.float32))
    b2 = np.ascontiguousarray(np.asarray(inputs["head_b2"], dtype=np.float32)
                              .reshape(C1 // P, P, 1))
    cur_iter = int(inputs["cur_iter"])
    gate = max((10000 - cur_iter) / 10000.0, 0.0)

    nc = build_nc(gate)
    in_maps = []
    for i in range(NCORES):
        in_maps.append({
            "x0": np.ascontiguousarray(x0[i * BL:(i + 1) * BL]),
            "vq0": vq0, "vq1": vq1,
            "head_w1": w1, "head_b1": b1, "head_w2": w2, "head_b2": b2,
        })
    res = run_bass_kernel_spmd(nc, in_maps, core_ids=list(range(NCORES)))
    global LAST_EXEC_NS
    LAST_EXEC_NS = res.exec_time_ns
    r = res.results

    def cat(name):
        return np.concatenate([r[i][name] for i in range(NCORES)], axis=0)

    x3 = cat("x3").reshape(B, C1, H, W)
    qx0 = cat("qx0").reshape(B, C, H, W)
    qx3 = cat("qx3").reshape(B, C1, H, W)
    a0 = cat("a0").reshape(B, K0, H, W)
    a3 = cat("a3").reshape(B, K1, H, W)
    d0 = cat("d0")
    d3 = cat("d3")
    return (x3, qx0, qx3, a0, a3, d0, d3)


# revision 17
# speedup vs baseline: 156.4789x; 156.4789x over previous
"""Trainium2 Bass kernel for nn_DULLI_21869973471277 (vq_codebook).

Data-parallel over batch across 8 NeuronCores. Each core handles 4 of 32
batch items (6400 of 51200 tokens):

  VQ0: d0 = |x|^2 + |e|^2 - 2 x.e  (k=2048, c=64) -> softmax(-d0/0.1) ->
       qx0 = gate*x + (1-gate) p0 @ E0
  head: x3 = W2 relu(W1 qx0 + b1) + b2   (1x1 convs, c 64->256->256)
  VQ1: same with k=512, c=256 -> qx3, a3, d3

Layouts: distances computed in [token, k] tiles (tokens on partitions) so
softmax reduces along the free axis; probabilities are PE-transposed into
[k, token] blocks which are simultaneously the a0/a3 output layout and the
lhs-contraction layout for q = p @ E.

All distance/feature matmuls use an fp16 hi/lo split (v = hi + lo, each
fp16): 3 bf16-speed matmuls replace one fp32 matmul (4x slower per row)
while keeping ~2^-22 relative error, which the T=0.1 softmax demands
(plain fp16/bf16/fp32r all fail: a d-error of eps becomes a factor
exp(10*eps) on the probabilities). The |x|^2-style norm rows ride in the
same augmented contraction and only need loose (distance-output) accuracy,
so they stay single fp16 rows; the |e|^2 rows vary along k and get hi+lo.
"""
from contextlib import ExitStack

import numpy as np

import concourse.bass as bass
import concourse.mybir as mybir
import concourse.tile as tile
from concourse import bacc
from concourse.bass_utils import run_bass_kernel_spmd
from concourse.masks import make_identity

F32 = mybir.dt.float32
F16 = mybir.dt.float16
AF = mybir.ActivationFunctionType
AX = mybir.AxisListType
ALU = mybir.AluOpType

B, C, H, W = 32, 64, 40, 40
HW = H * W                      # 1600 tokens per batch item
K0, C1, K1 = 2048, 256, 512
NCORES = 8
BL = B // NCORES                # 4 batch items per core
NL = BL * HW                    # 6400 tokens per core
TEMP = 0.1
P = 128
LAST_EXEC_NS = None
SUP = 256                       # super-tile: 2 sub-tiles of 128 tokens


def _pieces(tok0, n, hw):
    """Split core-local token range [tok0, tok0+n) into per-batch-item
    (b, hw0, col_off, width) pieces for strided output DMAs."""
    out = []
    t = tok0
    while t < tok0 + n:
        b = t // hw
        hw0 = t % hw
        w = min(hw - hw0, tok0 + n - t)
        out.append((b, hw0, t - tok0, w))
        t += w
    return out


def _split16(nc, hi, lo, src):
    """hi = fp16(src); lo = fp16(src - hi). src f32, hi/lo fp16 APs."""
    nc.vector.tensor_copy(hi, src)
    nc.vector.tensor_sub(lo, src, hi)


def build_nc(gate, bl=BL, hw=HW, reps=1, skew_mode=0):
    nl = bl * hw
    assert nl % SUP == 0
    nsup = nl // SUP
    g1 = 1.0 - gate
    tK0 = K0 // P               # 16 k-chunks for VQ0
    tK1 = K1 // P               # 4 k-chunks for VQ1
    nC1 = C1 // P               # 2 channel chunks for c=256

    nc = bacc.Bacc()
    x0p = nc.declare_dram_parameter("x0", [bl, C, hw], F32, isOutput=False)
    vq0p = nc.declare_dram_parameter("vq0", [K0, C], F32, isOutput=False)
    vq1p = nc.declare_dram_parameter("vq1", [K1, C1], F32, isOutput=False)
    w1p = nc.declare_dram_parameter("head_w1", [C1, C], F32, isOutput=False)
    b1p = nc.declare_dram_parameter("head_b1", [nC1, P, 1], F32, isOutput=False)
    w2p = nc.declare_dram_parameter("head_w2", [C1, C1], F32, isOutput=False)
    b2p = nc.declare_dram_parameter("head_b2", [nC1, P, 1], F32, isOutput=False)

    x3p = nc.declare_dram_parameter("x3", [bl, C1, hw], F32, isOutput=True)
    qx0p = nc.declare_dram_parameter("qx0", [bl, C, hw], F32, isOutput=True)
    qx3p = nc.declare_dram_parameter("qx3", [bl, C1, hw], F32, isOutput=True)
    a0p = nc.declare_dram_parameter("a0", [bl, K0, hw], F32, isOutput=True)
    a3p = nc.declare_dram_parameter("a3", [bl, K1, hw], F32, isOutput=True)
    d0p = nc.declare_dram_parameter("d0", [nl, K0], F32, isOutput=True)
    d3p = nc.declare_dram_parameter("d3", [nl, K1], F32, isOutput=True)

    with tile.TileContext(nc) as tc, ExitStack() as ctx:
        pers = ctx.enter_context(tc.tile_pool(name="pers", bufs=1))
        ps = ctx.enter_context(tc.tile_pool(name="ps", bufs=1, space="PSUM"))

        ident = pers.tile([P, P], F32)
        make_identity(nc, ident)
        ones_c = pers.tile([C, 1], F32)
        nc.gpsimd.memset(ones_c, 1.0)
        ones_p = pers.tile([P, 1], F32)
        nc.gpsimd.memset(ones_p, 1.0)
        ones_p16 = pers.tile([P, 1], F16)
        nc.gpsimd.memset(ones_p16, 1.0)

        # ---- persistent operand tiles -------------------------------------
        xa_hi = pers.tile([C + 2, nl], F16)   # rows: x | 1 | |x|^2
        xa_lo = pers.tile([C + 2, nl], F16)
        v0r_hi = pers.tile([C + 2, K0], F16)  # rows: -2 E^T | |e|^2 | 1
        v0r_lo = pers.tile([C + 2, K0], F16)
        v0g_hi = pers.tile([P, tK0, C], F16)  # (1-gate) * E, natural layout
        v0g_lo = pers.tile([P, tK0, C], F16)
        v1r_hi = [pers.tile([P, K1], F16, name=f"v1rh{g}", tag=f"v1rh{g}") for g in range(nC1)]
        v1r_lo = [pers.tile([P, K1], F16, name=f"v1rl{g}", tag=f"v1rl{g}") for g in range(nC1)]
        v1aug = pers.tile([3, K1], F16)       # rows: 1 | |e1|^2 hi | |e1|^2 lo
        xn_row = pers.tile([1, nl], F16)      # staging for the |x|^2 row
        v1g_hi = pers.tile([P, tK1, C1], F16)
        w1t = pers.tile([C, C1], F32)
        w2t = [pers.tile([P, C1], F32, name=f"w2t{g}", tag=f"w2t{g}") for g in range(nC1)]
        b1t = [pers.tile([P, 1], F32, name=f"b1t{g}", tag=f"b1t{g}") for g in range(nC1)]
        b2t = [pers.tile([P, 1], F32, name=f"b2t{g}", tag=f"b2t{g}") for g in range(nC1)]
        x3aug = pers.tile([3, nl], F16)       # rows: |x3|^2 | 1 | 1

        nc.gpsimd.memset(xa_hi[C:C + 2, :], 1.0)   # row C+1 redone via DMA
        nc.gpsimd.memset(xa_lo[C:C + 2, :], 0.0)
        nc.gpsimd.memset(v0r_hi[C:C + 2, :], 1.0)  # row C overwritten below
        nc.gpsimd.memset(v0r_lo[C:C + 2, :], 0.0)
        nc.gpsimd.memset(v1aug, 1.0)               # rows 1,2 redone via DMA
        nc.gpsimd.memset(x3aug, 1.0)               # row 0 rewritten per-super

        for g in range(nC1):
            nc.sync.dma_start(out=b1t[g], in_=b1p[g])
            nc.sync.dma_start(out=b2t[g], in_=b2p[g])

        # ---- setup: transposes, norms, fp16 splits ------------------------
        with tc.tile_pool(name="setup", bufs=1) as sp:
            x_f = sp.tile([C, nl], F32)
            for b in range(bl):
                nc.sync.dma_start(out=x_f[:, b * hw:(b + 1) * hw], in_=x0p[b])
            _split16(nc, xa_hi[0:C, :], xa_lo[0:C, :], x_f)
            for cstart in range(0, nl, 512):
                w = min(512, nl - cstart)
                xsq = sp.tile([C, 512], F32, tag="xsq")
                nc.vector.tensor_mul(xsq[:, :w], x_f[:, cstart:cstart + w],
                                     x_f[:, cstart:cstart + w])
                ps_r = ps.tile([1, 512], F32, tag="misc")
                nc.tensor.matmul(ps_r[:, :w], ones_c, xsq[:, :w],
                                 start=True, stop=True)
                nc.scalar.copy(xn_row[:, cstart:cstart + w], ps_r[:, :w])

            nc.sync.dma_start(out=xa_hi[C + 1:C + 2, :], in_=xn_row)

            # VQ0 codebook: natural load, transpose to [c, k], scale -2
            v0nat = sp.tile([P, tK0, C], F32)
            nc.sync.dma_start(out=v0nat,
                              in_=vq0p.rearrange("(t p) c -> p t c", p=P))
            v0t = sp.tile([C, K0], F32)     # holds -2 E^T
            for t in range(tK0):
                ps_t = ps.tile([P, P], F32, tag="pt")
                nc.tensor.transpose(ps_t[:C, :], v0nat[:, t, :], ident)
                nc.vector.tensor_scalar_mul(v0t[:, t * P:(t + 1) * P],
                                            ps_t[:C, :], -2.0)
            _split16(nc, v0r_hi[0:C, :], v0r_lo[0:C, :], v0t)
            en_hi0 = sp.tile([1, K0], F16)
            en_lo0 = sp.tile([1, K0], F16)
            for cstart in range(0, K0, 512):
                cs = slice(cstart, cstart + 512)
                esq = sp.tile([C, 512], F32, tag="esq")
                nc.vector.tensor_mul(esq, v0t[:, cs], v0t[:, cs])
                ps_r = ps.tile([1, 512], F32, tag="misc")
                nc.tensor.matmul(ps_r, ones_c, esq, start=True, stop=True)
                en_f = sp.tile([1, 512], F32, tag="enf")
                nc.vector.tensor_scalar_mul(en_f, ps_r, 0.25)
                nc.vector.tensor_copy(en_hi0[:, cs], en_f)
                nc.vector.tensor_sub(en_lo0[:, cs], en_f, en_hi0[:, cs])
            nc.sync.dma_start(out=v0r_hi[C:C + 1, :], in_=en_hi0)
            nc.sync.dma_start(out=v0r_lo[C:C + 1, :], in_=en_lo0)

            vgf = sp.tile([P, tK0, C], F32)
            nc.vector.tensor_scalar_mul(vgf, v0nat, g1)
            _split16(nc, v0g_hi, v0g_lo, vgf)

            # VQ1 codebook
            v1nat = sp.tile([P, tK1, C1], F32)
            nc.sync.dma_start(out=v1nat,
                              in_=vq1p.rearrange("(t p) c -> p t c", p=P))
            v1t = [sp.tile([P, K1], F32, name=f"v1t{g}", tag=f"v1t{g}") for g in range(nC1)]
            for t in range(tK1):
                for g in range(nC1):
                    ps_t = ps.tile([P, P], F32, tag="pt")
                    nc.tensor.transpose(ps_t, v1nat[:, t, g * P:(g + 1) * P], ident)
                    nc.vector.tensor_scalar_mul(v1t[g][:, t * P:(t + 1) * P],
                                                ps_t, -2.0)
            for g in range(nC1):
                _split16(nc, v1r_hi[g], v1r_lo[g], v1t[g])
            ps_r = ps.tile([1, 512], F32, tag="misc")
            for g in range(nC1):
                esq = sp.tile([P, K1], F32, tag="esq1")
                nc.vector.tensor_mul(esq, v1t[g], v1t[g])
                nc.tensor.matmul(ps_r, ones_p, esq, start=(g == 0),
                                 stop=(g == nC1 - 1))
            en_f = sp.tile([1, 512], F32, tag="enf")
            nc.vector.tensor_scalar_mul(en_f, ps_r, 0.25)
            en1_hi = sp.tile([1, 512], F16)
            en1_lo = sp.tile([1, 512], F16)
            nc.vector.tensor_copy(en1_hi, en_f)
            nc.vector.tensor_sub(en1_lo, en_f, en1_hi)
            nc.sync.dma_start(out=v1aug[1:2, :], in_=en1_hi)
            nc.sync.dma_start(out=v1aug[2:3, :], in_=en1_lo)

            vg1f = sp.tile([P, tK1, C1], F32)
            nc.vector.tensor_scalar_mul(vg1f, v1nat, g1)
            nc.vector.tensor_copy(v1g_hi, vg1f)

            # head weights: transpose W1 (256,64) -> [64, 256], W2 (256,256)
            w1nat = sp.tile([P, nC1, C], F32)
            nc.sync.dma_start(out=w1nat,
                              in_=w1p.rearrange("(t p) c -> p t c", p=P))
            for t in range(nC1):
                ps_t = ps.tile([P, P], F32, tag="pt")
                nc.tensor.transpose(ps_t[:C, :], w1nat[:, t, :], ident)
                nc.scalar.copy(w1t[:, t * P:(t + 1) * P], ps_t[:C, :])
            w2nat = sp.tile([P, nC1, C1], F32)
            nc.sync.dma_start(out=w2nat,
                              in_=w2p.rearrange("(t p) c -> p t c", p=P))
            for t in range(nC1):
                for g in range(nC1):
                    ps_t = ps.tile([P, P], F32, tag="pt")
                    nc.tensor.transpose(ps_t, w2nat[:, t, g * P:(g + 1) * P], ident)
                    nc.scalar.copy(w2t[g][:, t * P:(t + 1) * P], ps_t)

        # ---- main loop ----------------------------------------------------
        work = ctx.enter_context(tc.tile_pool(name="work", bufs=1))
        dbuf = ctx.enter_context(tc.tile_pool(name="dbuf", bufs=2))

        rep_loop = tc.For_i(0, reps, 1) if reps > 1 else None
        if rep_loop is not None:
            ctx.enter_context(rep_loop)

        def stage_a(s):
            """VQ0 distance/softmax/transposes + qx0 + head for super s."""
            t0 = s * SUP
            pt_hi = work.tile([P, tK0, SUP], F16, tag="pt_hi", bufs=2, name="pt_hi")
            pt_lo = work.tile([P, tK0, SUP], F16, tag="pt_lo", bufs=2, name="pt_lo")

            for sub in range(2):
                tok = t0 + sub * P
                sl = slice(tok, tok + P)
                ps_d = ps.tile([P, K0], F32, tag="d0", name="ps_d")
                for lhs, rhs, st, sp_ in ((xa_hi, v0r_hi, True, False),
                                          (xa_hi, v0r_lo, False, False),
                                          (xa_lo, v0r_hi, False, True)):
                    for j in range(K0 // 512):
                        js = slice(j * 512, (j + 1) * 512)
                        nc.tensor.matmul(ps_d[:, js], lhs[:, sl], rhs[:, js],
                                         start=st, stop=sp_)
                # row-min (DVE, from psum) runs in parallel with the psum->SBUF
                # copy (ACT); exp then reads the SBUF copy so the psum frees
                # right after the copy.
                d_sb = dbuf.tile([P, K0], F32, tag="d_sb", name="d_sb")
                nc.scalar.copy(d_sb[:, 0:K0 // 2], ps_d[:, 0:K0 // 2])
                nc.scalar.copy(d_sb[:, K0 // 2:], ps_d[:, K0 // 2:])
                nc.sync.dma_start(out=d0p[sl, :], in_=d_sb)
                rmn = dbuf.tile([P, 1], F32, tag="rmn", name="rmn")
                nc.vector.tensor_reduce(rmn, d_sb, axis=AX.X, op=ALU.min,
                                        negate=True)
                bias10 = dbuf.tile([P, 1], F32, tag="b10", name="bias10")
                nc.vector.tensor_scalar_mul(bias10, rmn, -1.0 / TEMP)
                e0 = dbuf.tile([P, K0], F32, tag="e0", name="e0")
                rs = dbuf.tile([P, 1], F32, tag="rs", name="rs")
                nc.scalar.activation(e0, d_sb, AF.Exp, bias=bias10,
                                     scale=-1.0 / TEMP, accum_out=rs)
                rinv = dbuf.tile([P, 1], F32, tag="rinv", name="rinv")
                nc.vector.reciprocal(rinv, rs)
                nc.vector.tensor_scalar_mul(e0, e0, rinv)   # e0 <- p0
                ssl = slice(sub * P, (sub + 1) * P)
                for j4 in range(0, tK0, 4):
                    ps_t4 = ps.tile([P, 4, P], F32, tag="pt", name="ps_t4")
                    for jj in range(4):
                        j = j4 + jj
                        nc.tensor.transpose(ps_t4[:, jj, :],
                                            e0[:, j * P:(j + 1) * P], ident)
                    nc.scalar.copy(pt_hi[:, j4:j4 + 4, ssl], ps_t4)
                    nc.vector.tensor_sub(pt_lo[:, j4:j4 + 4, ssl], ps_t4,
                                         pt_hi[:, j4:j4 + 4, ssl])

            pcs = _pieces(t0, SUP, hw)
            for (b, hw0, co, w) in pcs:
                nc.gpsimd.dma_start(
                    out=a0p[b].rearrange("(j p) hw -> p j hw", p=P)[:, :, hw0:hw0 + w],
                    in_=pt_hi[:, :, co:co + w])

            # qx0 = gate*x + (1-gate) p0 @ E0   (computed transposed [C, SUP])
            ps_q = ps.tile([C, SUP], F32, tag="misc", name="ps_q")
            for t in range(tK0):
                nc.tensor.matmul(ps_q, v0g_hi[:, t, :], pt_hi[:, t, :],
                                 start=(t == 0), stop=False)
                nc.tensor.matmul(ps_q, v0g_hi[:, t, :], pt_lo[:, t, :],
                                 start=False, stop=False)
                nc.tensor.matmul(ps_q, v0g_lo[:, t, :], pt_hi[:, t, :],
                                 start=False, stop=(t == tK0 - 1))
            ssup = slice(t0, t0 + SUP)
            qx0 = dbuf.tile([C, SUP], F32, tag="qx0", name="qx0")
            nc.vector.tensor_add(qx0, xa_hi[0:C, ssup], xa_lo[0:C, ssup])
            nc.vector.tensor_scalar_mul(qx0, qx0, gate)
            nc.vector.tensor_add(qx0, qx0, ps_q)
            for (b, hw0, co, w) in pcs:
                nc.sync.dma_start(out=qx0p[b, :, hw0:hw0 + w],
                                  in_=qx0[:, co:co + w])

            # head: y1 = relu(W1 qx0 + b1); x3 = W2 y1 + b2
            y1 = [dbuf.tile([P, SUP], F32, name=f"y1_{g}", tag=f"y1_{g}") for g in range(nC1)]
            for g in range(nC1):
                ps_y = ps.tile([P, SUP], F32, tag="misc", name="ps_y")
                nc.tensor.matmul(ps_y, w1t[:, g * P:(g + 1) * P], qx0,
                                 start=True, stop=True)
                nc.scalar.activation(y1[g], ps_y, AF.Relu, bias=b1t[g])
            x3t = [dbuf.tile([P, SUP], F32, name=f"x3_{o}", tag=f"x3_{o}") for o in range(nC1)]
            x3hi = [dbuf.tile([P, SUP], F16, name=f"x3h{o}", tag=f"x3h{o}") for o in range(nC1)]
            x3lo = [dbuf.tile([P, SUP], F16, name=f"x3l{o}", tag=f"x3l{o}") for o in range(nC1)]
            for o in range(nC1):
                ps_x = ps.tile([P, SUP], F32, tag="misc", name="ps_x")
                for g in range(nC1):
                    nc.tensor.matmul(ps_x, w2t[g][:, o * P:(o + 1) * P], y1[g],
                                     start=(g == 0), stop=(g == nC1 - 1))
                nc.scalar.activation(x3t[o], ps_x, AF.Identity, bias=b2t[o])
                for (b, hw0, co, w) in pcs:
                    nc.sync.dma_start(out=x3p[b, o * P:(o + 1) * P, hw0:hw0 + w],
                                      in_=x3t[o][:, co:co + w])
                _split16(nc, x3hi[o], x3lo[o], x3t[o])

            # |x3|^2 row (fp16 accuracy is plenty: uniform across k)
            ps_r = ps.tile([1, SUP], F32, tag="misc", name="ps_r")
            for o in range(nC1):
                sq = dbuf.tile([P, SUP], F16, tag="sq3", name="sq")
                nc.vector.tensor_mul(sq, x3t[o], x3t[o])
                nc.tensor.matmul(ps_r, ones_p16, sq, start=(o == 0),
                                 stop=(o == nC1 - 1))
            nc.scalar.copy(x3aug[0:1, ssup], ps_r)
            return dict(t0=t0, pcs=pcs, x3t=x3t, x3hi=x3hi, x3lo=x3lo)

        def stage_b(st):
            """VQ1 for the super described by st (runs one super behind)."""
            t0, pcs = st["t0"], st["pcs"]
            x3t, x3hi, x3lo = st["x3t"], st["x3hi"], st["x3lo"]
            pt3_hi = work.tile([P, tK1, SUP], F16, tag="pt3_hi", bufs=2,
                               name="pt3_hi")
            for sub in range(2):
                tok = t0 + sub * P
                sl = slice(tok, tok + P)
                ssl = slice(sub * P, (sub + 1) * P)
                ps_d3 = ps.tile([P, K1], F32, tag="misc", name="ps_d3")
                for g in range(nC1):
                    nc.tensor.matmul(ps_d3, x3hi[g][:, ssl], v1r_hi[g],
                                     start=(g == 0), stop=False)
                    nc.tensor.matmul(ps_d3, x3hi[g][:, ssl], v1r_lo[g],
                                     start=False, stop=False)
                for g in range(nC1):
                    nc.tensor.matmul(ps_d3, x3lo[g][:, ssl], v1r_hi[g],
                                     start=False, stop=False)
                nc.tensor.matmul(ps_d3, x3aug[:, sl], v1aug,
                                 start=False, stop=True)
                d_sb = dbuf.tile([P, K1], F32, tag="d3_sb", name="d_sb")
                nc.scalar.copy(d_sb, ps_d3)
                nc.sync.dma_start(out=d3p[sl, :], in_=d_sb)
                rmn = dbuf.tile([P, 1], F32, tag="rmn3", name="rmn")
                nc.vector.tensor_reduce(rmn, d_sb, axis=AX.X, op=ALU.min,
                                        negate=True)
                bias10 = dbuf.tile([P, 1], F32, tag="b103", name="bias10")
                nc.vector.tensor_scalar_mul(bias10, rmn, -1.0 / TEMP)
                e3 = dbuf.tile([P, K1], F32, tag="e3", name="e3")
                rs = dbuf.tile([P, 1], F32, tag="rs3", name="rs")
                nc.scalar.activation(e3, d_sb, AF.Exp, bias=bias10,
                                     scale=-1.0 / TEMP, accum_out=rs)
                rinv = dbuf.tile([P, 1], F32, tag="rinv3", name="rinv")
                nc.vector.reciprocal(rinv, rs)
                nc.vector.tensor_scalar_mul(e3, e3, rinv)   # e3 <- p3
                ps_t4 = ps.tile([P, 4, P], F32, tag="pt", name="ps_t4")
                for t in range(tK1):
                    nc.tensor.transpose(ps_t4[:, t, :],
                                        e3[:, t * P:(t + 1) * P], ident)
                nc.scalar.copy(pt3_hi[:, :, ssl], ps_t4)

            for (b, hw0, co, w) in pcs:
                nc.gpsimd.dma_start(
                    out=a3p[b].rearrange("(t p) hw -> p t hw", p=P)[:, :, hw0:hw0 + w],
                    in_=pt3_hi[:, :, co:co + w])

            # qx3 = gate*x3 + (1-gate) p3 @ E1
            for o in range(nC1):
                ps_q3 = ps.tile([P, SUP], F32, tag="misc", name="ps_q3")
                for t in range(tK1):
                    nc.tensor.matmul(ps_q3, v1g_hi[:, t, o * P:(o + 1) * P],
                                     pt3_hi[:, t, :],
                                     start=(t == 0), stop=(t == tK1 - 1))
                qx3 = dbuf.tile([P, SUP], F32, tag=f"qx3_{o}", name=f"qx3_{o}")
                nc.vector.tensor_scalar_mul(qx3, x3t[o], gate)
                nc.vector.tensor_add(qx3, qx3, ps_q3)
                for (b, hw0, co, w) in pcs:
                    nc.sync.dma_start(out=qx3p[b, o * P:(o + 1) * P, hw0:hw0 + w],
                                      in_=qx3[:, co:co + w])

        # one-super skew: stage_b(s-1) is emitted after stage_a(s) so every
        # engine's in-order queue holds ready (input-complete) VQ1 work to
        # fill the dependency stalls of super s's VQ0 chain.
        skew = skew_mode if nsup > 1 else 0
        prev = None
        for s in range(nsup):
            cur = stage_a(s)
            if skew == 0:
                stage_b(cur)
            else:
                if prev is not None:
                    stage_b(prev)
                prev = cur
        if skew and prev is not None:
            stage_b(prev)

    nc.compile()
    return nc


def kernel(**inputs):
    x0 = np.ascontiguousarray(np.asarray(inputs["x0"], dtype=np.float32)
                              .reshape(B, C, HW))
    vq0 = np.ascontiguousarray(np.asarray(inputs["vq0"], dtype=np.float32))
    vq1 = np.ascontiguousarray(np.asarray(inputs["vq1"], dtype=np.float32))
    w1 = np.ascontiguousarray(np.asarray(inputs["head_w1"], dtype=np.float32))
    b1 = np.ascontiguousarray(np.asarray(inputs["head_b1"], dtype=np.float32)
                              .reshape(C1 // P, P, 1))
    w2 = np.ascontiguousarray(np.asarray(inputs["head_w2"], dtype=np.float32))
    b2 = np.ascontiguousarray(np.asarray(inputs["head_b2"], dtype=np.float32)
                              .reshape(C1 // P, P, 1))
    cur_iter = int(inputs["cur_iter"])
    gate = max((10000 - cur_iter) / 10000.0, 0.0)

    nc = build_nc(gate)
    in_maps = []
    for i in range(NCORES):
        in_maps.append({
            "x0": np.ascontiguousarray(x0[i * BL:(i + 1) * BL]),
            "vq0": vq0, "vq1": vq1,
            "head_w1": w1, "head_b1": b1, "head_w2": w2, "head_b2": b2,
        })
    res = run_bass_kernel_spmd(nc, in_maps, core_ids=list(range(NCORES)))
    global LAST_EXEC_NS
    LAST_EXEC_NS = res.exec_time_ns
    r = res.results

    def cat(name):
        return np.concatenate([r[i][name] for i in range(NCORES)], axis=0)

    x3 = cat("x3").reshape(B, C1, H, W)
    qx0 = cat("qx0").reshape(B, C, H, W)
    qx3 = cat("qx3").reshape(B, C1, H, W)
    a0 = cat("a0").reshape(B, K0, H, W)
    a3 = cat("a3").reshape(B, K1, H, W)
    d0 = cat("d0")
    d3 = cat("d3")
    return (x3, qx0, qx3, a0, a3, d0, d3)
